# revision 13
# baseline (speedup 1.0000x reference)
"""Cross-modal attention Trainium2 kernel (v5).

Reference computation (all 1x1 convs + folded eval-mode BN):
  q = BN(Wq @ rgb), k = BN(Wk @ edge), v = BN(Wv @ edge)
  attn = softmax(q^T k) per head; xx = relu(attn @ v); out = BN(Wp @ xx)

Shapes: B=2, C=256, H=W=64 (N=4096), heads=8, key_dim=16, d=32.

Sharding: 8 cores = (batch b in {0,1}) x (query-slice qs in {0..3}, 1024
queries each). Each core computes K/V projections for the full N of its
batch (cheap) and attention + output projection for its query slice; the
host concatenates slices. No collectives.

v5 changes vs the 198us baseline:
  - fp16 input pipeline: rgb/edge and the projection weights ship as
    fp16 (half the DMA bytes -> input wire time ~11us instead of ~21us;
    group 0 was DMA-throttled). fp16's 10-bit mantissa matches f32r
    matmul precision, and all downstream score/exp/AV math is unchanged
    (kk/qq stay f32r from the f32 PSUM).
  - Bias algebra: q-bias and bk.bq are softmax-invariant (constant along
    the key axis) and dropped; the k-bias term bq.k_hat[m] is folded as a
    17th contraction row (kk row 32j+16 = Wk_f^T bq_h projected, qq row
    32j+16 = 1.0 via DMA, re-DMA'd after each projection-copy half).
    Projection PSUM->SBUF moves become pure copies placeable on either
    ACT or DVE.
  - Exp engine pattern retuned for the shifted fixed loads.

Per-core dataflow (scores kept transposed so softmax-sum and the AV
contraction both run on the m axis): see baseline notes. Exp split:
ACT true exp / DVE Schraudolph bitwise 2^x (~+-3% multiplicative error
that cancels in softmax normalization). GPSIMD cannot touch PSUM, so
Pool only carries the SBUF-side per-head xnm scales.

PSUM (8 banks): scp 3 x [128,1024]f32 (2 banks each, depth-3 QKT->exp
rotation; in-stream projection psums transiently borrow scp zones);
avp [128,264]f32; fixt [128,512]f32 (vto double-buffer slots, later
outproj psums and transpose outputs).
"""

import sys

for p in ("/opt/trn_rl_repo", "/opt/trn_rl_repo/concourse"):
    if p not in sys.path:
        sys.path.insert(0, p)

import numpy as np

import concourse.bass as bass
import concourse.mybir as mybir
import concourse.tile as tile
from concourse.bass_utils import run_bass_kernel_spmd

F32 = mybir.dt.float32
F32R = mybir.dt.float32r
F16 = mybir.dt.float16
BF16 = mybir.dt.bfloat16
I16 = mybir.dt.int16
AF = mybir.ActivationFunctionType
ALU = mybir.AluOpType

NUM_HEADS, KD, D = 8, 16, 32
B, C, H, W = 2, 256, 64, 64
N = H * W            # 4096 keys per batch
QCH = 1024           # queries per core
NMT = N // 128       # 32 m-tiles
HB = 33              # per-head AV block: 32 v-cols + 1 denominator col
KC = 17              # contraction rows per head: 16 kd + 1 bias row

# Schraudolph exp: bf16_bits(int16(s*SCH_C1 + SCH_C2)) ~= e^s.
SCH_C1 = 184.66496580927726     # 128 * log2(e)
SCH_C2 = 16248.6                # 127*128 minus mean-zeroing interp bias


def _mk_pat(nA, nD):
    """Evenly interleave nA 'A's and nD 'D's (Bresenham)."""
    out, a, d = [], 0, 0
    for i in range(nA + nD):
        if a * nD <= d * nA and a < nA:
            out.append("A")
            a += 1
        else:
            out.append("D")
            d += 1
    return out


# Per-group exp-engine pattern (32 tiles each). Group 0 skews toward ACT
# (DVE carries the vto adds early on); steady-state ~16/16.
GROUP_PATS = [_mk_pat(26, 6), _mk_pat(17, 15)] + [_mk_pat(16, 16)] * 6


def build_nc():
    nc = bass.Bass()

    rgb_s = nc.dram_tensor("rgb_s", [C, QCH], F16, kind="ExternalInput")
    edge = nc.dram_tensor("edge", [C, N], F16, kind="ExternalInput")
    w_qA = nc.dram_tensor("w_qA", [C, 128], F16, kind="ExternalInput")
    w_qB = nc.dram_tensor("w_qB", [C, 128], F16, kind="ExternalInput")
    w_kA = nc.dram_tensor("w_kA", [C, 128], F16, kind="ExternalInput")
    w_kB = nc.dram_tensor("w_kB", [C, 128], F16, kind="ExternalInput")
    w_v = nc.dram_tensor("w_v", [C, 256], F16, kind="ExternalInput")
    w_p = nc.dram_tensor("w_p", [256, C], BF16, kind="ExternalInput")
    b_v = nc.dram_tensor("b_v", [128, 256], F32, kind="ExternalInput")
    b_p = nc.dram_tensor("b_p", [C, 1], F32, kind="ExternalInput")
    ones_f = nc.dram_tensor("ones_f", [4, QCH], F32R, kind="ExternalInput")
    ident = nc.dram_tensor("ident", [128, 128], BF16, kind="ExternalInput")
    out = nc.dram_tensor("out", [C, QCH], F32, kind="ExternalOutput")

    with tile.TileContext(nc) as tc:
        with tc.tile_pool(name="const", bufs=1) as cp, \
             tc.tile_pool(name="data", bufs=1) as dp:
            wq = [cp.tile([128, 256], F16, name=f"wq{x}", tag=f"wq{x}") for x in "AB"]
            wk = [cp.tile([128, 256], F16, name=f"wk{x}", tag=f"wk{x}") for x in "AB"]
            wv = cp.tile([128, 512], F16, name="wv", tag="wv")
            wp = cp.tile([128, 512], BF16, name="wp", tag="wp")
            bv = cp.tile([128, 256], F32, name="bv", tag="bv")
            bp = cp.tile([128, 2], F32, name="bp", tag="bp")
            idn = cp.tile([128, 128], BF16, name="idn", tag="idn")
            zrow = cp.tile([1, 128], BF16, name="zrow", tag="zrow")

            nc.vector.memset(zrow[:], 0.0)

            rgb_sb = dp.tile([128, 2 * QCH], F16, name="rgb_sb", tag="rgb")
            edge_sb = [dp.tile([128, N], F16, name=f"edge{k}", tag=f"edge{k}") for k in range(2)]

            # Input DMA ordering: first-use order. kk chunk 0 needs wk +
            # edge cols 0-511; qq(0,0) needs wq + rgb cols 0-511.
            for k in range(2):
                nc.sync.dma_start(wk[0][:, 128 * k:128 * (k + 1)], w_kA[128 * k:128 * (k + 1), :])
                nc.scalar.dma_start(wq[0][:, 128 * k:128 * (k + 1)], w_qA[128 * k:128 * (k + 1), :])
            for k in range(2):      # first edge columns (kk chunk 0 dep)
                nc.scalar.dma_start(edge_sb[k][:, 0:512], edge[128 * k:128 * (k + 1), 0:512])
            for k in range(2):      # rgb first halves (qc=0 queries)
                nc.scalar.dma_start(rgb_sb[:, QCH * k:QCH * k + 512], rgb_s[128 * k:128 * (k + 1), 0:512])
            for k in range(2):
                nc.sync.dma_start(edge_sb[k][:, 512:1024], edge[128 * k:128 * (k + 1), 512:1024])
            for k in range(2):
                nc.sync.dma_start(wv[:, 256 * k:256 * (k + 1)], w_v[128 * k:128 * (k + 1), :])
            nc.sync.dma_start(bv[:], b_v[:])
            nc.sync.dma_start(bp[:, 0:1], b_p[0:128, :])
            nc.sync.dma_start(bp[:, 1:2], b_p[128:256, :])
            for k in range(2):
                nc.sync.dma_start(edge_sb[k][:, 1024:2048], edge[128 * k:128 * (k + 1), 1024:2048])
            for k in range(2):
                nc.sync.dma_start(rgb_sb[:, QCH * k + 512:QCH * (k + 1)], rgb_s[128 * k:128 * (k + 1), 512:1024])
            for k in range(2):
                nc.sync.dma_start(wk[1][:, 128 * k:128 * (k + 1)], w_kB[128 * k:128 * (k + 1), :])
                nc.sync.dma_start(wq[1][:, 128 * k:128 * (k + 1)], w_qB[128 * k:128 * (k + 1), :])
            for p in range(2, 4):
                for k in range(2):
                    nc.sync.dma_start(edge_sb[k][:, 1024 * p:1024 * (p + 1)],
                                      edge[128 * k:128 * (k + 1), 1024 * p:1024 * (p + 1)])
            for k in range(2):
                nc.sync.dma_start(wp[:, 256 * k:256 * (k + 1)], w_p[128 * k:128 * (k + 1), :])
            nc.sync.dma_start(idn[:], ident[:])

            qq = [dp.tile([128, QCH], F32R, name=f"qq{x}", tag=f"qq{x}") for x in "AB"]
            kk = [dp.tile([128, N], F32R, name=f"kk{x}", tag=f"kk{x}") for x in "AB"]
            vto = dp.tile([128, NMT * 8 * HB], BF16, name="vto", tag="vto")
            # denominator columns (33rd of each head block) are constant 1.0
            nc.vector.memset(
                vto[:].rearrange("p (m c) -> p m c", c=HB)[:, :, 32], 1.0)
            outb = [dp.tile([128, QCH], F32, name=f"outb{k}", tag=f"outb{k}") for k in range(2)]
            xxt = [dp.tile([128, 512], BF16, name=f"xxt{k}", tag=f"xxt{k}") for k in range(2)]

            # ---- pools ----
            scp = tc.alloc_tile_pool(name="scp", bufs=3, space="PSUM")
            avp = tc.alloc_tile_pool(name="avp", bufs=1, space="PSUM")
            psv = tc.alloc_tile_pool(name="psv", bufs=1, space="PSUM")
            fixt = psv.tile([128, 512], F32, name="fixt", tag="v")
            ep = tc.alloc_tile_pool(name="exp", bufs=12)
            sp = tc.alloc_tile_pool(name="stg", bufs=6)

            # ACT warmup: keep the first real exp single-wait.
            actw = dp.tile([1, 1], BF16, name="actw", tag="actw")
            nc.scalar.activation(actw[:], zrow[0:1, 0:1], AF.Exp)

            def copy_ps(dst, ps, eng):
                # PSUM->SBUF projection move: ACT Identity or DVE copy.
                if eng == "A":
                    nc.scalar.activation(dst, ps, AF.Identity)
                else:
                    nc.vector.tensor_copy(dst, ps)

            def ones_dma(x, j):
                # rewrite the const-1 rows (32g+16) of qq[x] cols 512j..
                # (the projection copy clobbers all 128 partitions).
                nc.sync.dma_start(
                    qq[x].rearrange("(g p) q -> g p q", p=32)[:, 16:17, 512 * j:512 * (j + 1)],
                    ones_f[:, 512 * j:512 * (j + 1)].unsqueeze(1))

            def proj_qq(x, j, eng, ps=None):
                ps = ps if ps is not None else scp.tile([128, 512], F32, name="ps_q", tag="w")
                for k in range(2):
                    nc.tensor.matmul(
                        ps, wq[x][:, 128 * k:128 * (k + 1)],
                        rgb_sb[:, QCH * k + 512 * j:QCH * k + 512 * (j + 1)],
                        start=(k == 0), stop=(k == 1))
                copy_ps(qq[x][:, 512 * j:512 * (j + 1)], ps, eng)
                ones_dma(x, j)

            def proj_kk(x, j, eng, ps=None):
                ps = ps if ps is not None else scp.tile([128, 512], F32, name="ps_k", tag="w")
                for k in range(2):
                    nc.tensor.matmul(
                        ps, wk[x][:, 128 * k:128 * (k + 1)],
                        edge_sb[k][:, 512 * j:512 * (j + 1)],
                        start=(k == 0), stop=(k == 1))
                copy_ps(kk[x][:, 512 * j:512 * (j + 1)], ps, eng)

            def proj_vto(mt):
                ps = fixt[:, 256 * (mt % 2):256 * (mt % 2) + 256]
                for k in range(2):
                    nc.tensor.matmul(
                        ps, edge_sb[k][:, 128 * mt:128 * (mt + 1)],
                        wv[:, 256 * k:256 * (k + 1)],
                        start=(k == 0), stop=(k == 1))
                nc.vector.tensor_add(
                    vto[:, 8 * HB * mt:8 * HB * (mt + 1)].rearrange(
                        "p (h c) -> p h c", c=HB)[:, :, 0:32],
                    ps.rearrange("p (h c) -> p h c", c=32), bv[:])

            def transposes(x):
                xnm_x = xnm_tiles[x]
                for s in range(4):
                    tp = fixt[:, 64 * s:64 * (s + 1)].bitcast(BF16)
                    nc.tensor.transpose(tp, xnm_x[:, 128 * s:128 * (s + 1)], idn[:])
                    nc.vector.tensor_copy(xxt[x][:, 128 * s:128 * (s + 1)], tp)

            def outproj(qc):
                q0 = 512 * qc
                for ct in range(2):
                    ps = fixt[:]
                    for k in range(2):
                        nc.tensor.matmul(
                            ps, wp[:, 256 * k + 128 * ct:256 * k + 128 * (ct + 1)],
                            xxt[k][:], start=(k == 0), stop=(k == 1))
                    nc.scalar.activation(
                        outb[ct][:, q0:q0 + 512], ps,
                        AF.Identity, bias=bp[:, ct:ct + 1])
                    if qc == 1:
                        nc.sync.dma_start(
                            out[128 * ct:128 * (ct + 1), 512:1024],
                            outb[ct][:, 512:1024])

            # per-mtile PE-stream hooks: {(gi, mt): [fn, ...]}
            hooks = {}

            def add_hook(gi, mt, fn):
                hooks.setdefault((gi, mt), []).append(fn)

            # pre-loop: first kk chunk + first qq half, vto 0-3. The rest
            # of kk[0] rides gi0 hooks (paced by edge arrival, scp psums);
            # kk[1]/qq halves ride later hooks (fixt once vto is done).
            proj_kk(0, 0, "A")
            proj_qq(0, 0, "D")
            for i in range(4):
                proj_vto(i)
            for i in range(28):             # vto(4+i), 4 mtiles ahead of use
                add_hook(0, i, lambda mt=4 + i: proj_vto(mt))
            for c in range(1, 8):           # kk[0] chunk c feeds QKT(mt>=4c)
                add_hook(0, 4 * c - 2,
                         lambda c=c: proj_kk(0, c, "A" if c % 2 else "D"))
            add_hook(1, 24, lambda: proj_qq(1, 0, "D", fixt[:]))
            add_hook(3, 10, lambda: proj_qq(0, 1, "A", fixt[:]))
            add_hook(5, 14, lambda: proj_qq(1, 1, "D", fixt[:]))
            add_hook(1, 28, lambda: proj_kk(1, 0, "A", fixt[:]))
            for c in range(1, 8):
                add_hook(2, 4 * c - 2,
                         lambda c=c: proj_kk(1, c, "A" if c % 2 else "D", fixt[:]))

            def emit_exp(et, sc, eng):
                if eng == "A":
                    nc.scalar.activation(et, sc, AF.Exp)
                else:
                    nc.vector.tensor_scalar(
                        et.bitcast(I16), sc, SCH_C1, SCH_C2,
                        ALU.mult, ALU.add)

            GROUPS = [(qc, x, pr) for qc in (0, 1) for x in (0, 1) for pr in (0, 1)]
            # deferred PE-side epilogue hooks: group gi's xnm tile (x done
            # at odd gi) is transposed inside group gi+1's stream; the
            # output projection of qc=0 rides in group 4.
            add_hook(2, 10, lambda: transposes(0))
            add_hook(4, 10, lambda: transposes(1))
            add_hook(4, 16, lambda: outproj(0))
            add_hook(6, 10, lambda: transposes(0))

            def out_dma_qc0():
                for ct in range(2):
                    nc.sync.dma_start(out[128 * ct:128 * (ct + 1), 0:512],
                                      outb[ct][:, 0:512])
            add_hook(5, 8, out_dma_qc0)
            LAG = 4
            xnm_tiles = [None, None]
            pend = []       # (emit_av_fn, post_fn_or_None)

            def flush_one():
                fn, post = pend.pop(0)
                fn()
                if post is not None:
                    post()

            for gi, (qc, x, pr) in enumerate(GROUPS):
                q0 = 512 * qc
                pat = GROUP_PATS[gi]
                avt = avp.tile([128, 264], F32, name="av", tag="av")

                def make_av(avt, x, pr, mt, et):
                    def emit_av():
                        for j2 in range(2):
                            h = 4 * x + 2 * pr + j2
                            for s in range(4):
                                nc.tensor.matmul(
                                    avt[:, 66 * s + 33 * j2:66 * s + 33 * j2 + 33],
                                    et[:, 512 * j2 + 128 * s:512 * j2 + 128 * (s + 1)],
                                    vto[:, 8 * HB * mt + HB * h:8 * HB * mt + HB * (h + 1)],
                                    start=False, stop=(mt == NMT - 1))
                    return emit_av

                def make_epilogue(avt, gi, qc, x, pr):
                    def epilogue():
                        xxm = sp.tile([128, 264], F32, name="xxm", tag="xxm")
                        nc.scalar.activation(xxm[:], avt[:], AF.Relu)
                        rden = sp.tile([128, 8], F32, name="rden", tag="rden")
                        nc.vector.reciprocal(
                            rden[:],
                            xxm[:].rearrange("p (g c) -> p g c", c=33)[:, :, 32])
                        if pr == 0:
                            xnm_tiles[x] = sp.tile([128, 512], BF16,
                                                   name="xnm", tag="xnm")
                        xnm_x = xnm_tiles[x]
                        if gi == len(GROUPS) - 1:
                            # tail: DVE is idle; one strided op beats the
                            # serial Pool-launch chain
                            nc.vector.scalar_tensor_tensor(
                                xnm_x[:].rearrange(
                                    "p (s v u c) -> p s v u c",
                                    v=2, u=2, c=32)[:, :, pr, :, :],
                                xxm[:].rearrange(
                                    "p (s u c) -> p s u c",
                                    u=2, c=33)[:, :, :, 0:32],
                                1.0,
                                rden[:].rearrange("p (s u) -> p s u", u=2)
                                    .unsqueeze(3).broadcast_to([128, 4, 2, 32]),
                                ALU.mult, ALU.mult)
                        else:
                            for g in range(8):      # g = 2*s + j2
                                s, j2 = g // 2, g % 2
                                c0 = 128 * s + 64 * pr + 32 * j2
                                nc.gpsimd.tensor_scalar_mul(
                                    xnm_x[:, c0:c0 + 32],
                                    xxm[:, 33 * g:33 * g + 32],
                                    rden[:, g:g + 1])
                    return epilogue

                for mt in range(NMT):
                    for fn in hooks.get((gi, mt), []):
                        fn()
                    if mt == 6:
                        # whole-bank start=True clear of this group's AV
                        # accumulator; deferred here so it lands after the
                        # previous group's epilogue (relu) was emitted.
                        nc.tensor.matmul(avt[:], zrow[:], vto[0:1, 0:264],
                                         start=True, stop=False)
                    limit = 7 if mt < 8 else LAG
                    while len(pend) > limit:
                        flush_one()
                    sc = scp.tile([128, 1024], F32, name="sc", tag="w")
                    for j2 in range(2):
                        j = 2 * pr + j2
                        nc.tensor.matmul(
                            sc[:, 512 * j2:512 * (j2 + 1)],
                            kk[x][32 * j:32 * j + KC, 128 * mt:128 * (mt + 1)],
                            qq[x][32 * j:32 * j + KC, q0:q0 + 512],
                            start=True, stop=True,
                            tile_position=(32 * j, 0))
                    et = ep.tile([128, 1024], BF16, name="et", tag="et")
                    emit_exp(et[:], sc[:], pat[mt])
                    post = make_epilogue(avt, gi, qc, x, pr) if mt == NMT - 1 else None
                    pend.append((make_av(avt, x, pr, mt, et), post))

            # tail: flush remaining AVs (fires the last epilogue), then
            # the final transposes + output projection
            while pend:
                flush_one()
            transposes(1)
            outproj(1)

            for _p in (sp, ep, psv, avp, scp):
                _p.release()

    # walrus codegen accepts only ONE sync wait on compute instructions.
    # Drop self-engine waits (hardware-guaranteed in-order), then hoist
    # any remaining extras into same-engine NoOps.
    _self_prefix = {
        "EngineType.PE": "PE",
        "EngineType.Activation": "Activation",
        "EngineType.DVE": "DVE",
        "EngineType.Pool": "Pool",
        "EngineType.SP": "SP",
    }
    for f in nc.m.functions:
        for bb in f.blocks:
            for inst in bb.instructions:
                si = inst.sync_info
                if si is None or not si.on_wait or len(si.on_wait) < 2:
                    continue
                pref = _self_prefix.get(str(getattr(inst, "engine", "")), None)
                if pref is None:
                    continue
                kept = [w for w in si.on_wait
                        if not str(w.ant_name).startswith(pref)]
                if not kept or len(kept) == len(si.on_wait):
                    continue
                si.on_wait = kept

    uid = [0]
    for f in nc.m.functions:
        for bb in f.blocks:
            new_insts = []
            for inst in bb.instructions:
                si = inst.sync_info
                if si is not None and si.on_wait and len(si.on_wait) > 1:
                    for w in si.on_wait[:-1]:
                        uid[0] += 1
                        nop = mybir.InstNoOp(
                            name=f"I-waitsplit-{uid[0]}", ins=[], outs=[])
                        nop.engine = inst.engine
                        nop.sync_info = mybir.SyncInfo(
                            on_wait=[w], on_update=[])
                        new_insts.append(nop)
                    si.on_wait = [si.on_wait[-1]]
                new_insts.append(inst)
            bb.instructions = new_insts
    return nc


_CACHE = {}


def _prep_host(inputs):
    """Fold BN into weights; build head-split layouts shared by all cores."""
    import ml_dtypes
    f = np.float32
    f16 = np.float16
    Wq = (inputs["Wq"] * inputs["sq"][:, None]).astype(f)
    Wk = (inputs["Wk"] * inputs["sk"][:, None]).astype(f)
    Wv = (inputs["Wv"] * inputs["sv"][:, None]).astype(f)
    Wp = (inputs["Wp"] * inputs["sp"][:, None]).astype(f)
    bq = inputs["bq"].astype(f)

    def split_q(Wt):
        # Wt: [C, 128] (transposed folded Wq). Bias rows dropped
        # (q-bias term is constant along the softmax axis).
        o = []
        for g in range(2):
            Wx = np.zeros((C, 128), f)
            for j in range(4):
                h = 4 * g + j
                Wx[:, 32 * j:32 * j + KD] = Wt[:, KD * h:KD * (h + 1)]
            o.append(np.ascontiguousarray(Wx).astype(f16))
        return o

    def split_k(Wt):
        # col 32j+16 projects rk[m] = bq_h^T k_hat[m] (k-bias term folded
        # as a 17th contraction row; pairs with the const-1 qq row).
        o = []
        for g in range(2):
            Wx = np.zeros((C, 128), f)
            for j in range(4):
                h = 4 * g + j
                Wx[:, 32 * j:32 * j + KD] = Wt[:, KD * h:KD * (h + 1)]
                Wx[:, 32 * j + 16] = Wt[:, KD * h:KD * (h + 1)] @ bq[KD * h:KD * (h + 1)]
            o.append(np.ascontiguousarray(Wx).astype(f16))
        return o

    wqA, wqB = split_q(Wq.T.astype(f))
    wkA, wkB = split_k(Wk.T.astype(f))
    WvT = Wv.T.astype(f)                      # [C, 256] cols (h, d)
    ident_bf16 = np.eye(128, dtype=ml_dtypes.bfloat16)
    return dict(
        w_qA=wqA, w_qB=wqB, w_kA=wkA, w_kB=wkB,
        w_v=np.ascontiguousarray(WvT).astype(f16),
        w_p=np.ascontiguousarray(Wp.T).astype(ml_dtypes.bfloat16),
        b_v=np.ascontiguousarray(np.broadcast_to(inputs["bv"].astype(f), (128, 256))),
        b_p=inputs["bp"].astype(f).reshape(C, 1),
        ones_f=np.ones((4, QCH), f),
        ident=ident_bf16,
    )


def kernel(**inputs) -> np.ndarray:
    inputs = {k: np.asarray(v) for k, v in inputs.items()}
    if "nc" not in _CACHE:
        _CACHE["nc"] = build_nc()
    nc = _CACHE["nc"]

    shared = _prep_host(inputs)
    rgb = np.ascontiguousarray(inputs["rgb"].astype(np.float16).reshape(B, C, N))
    edge = np.ascontiguousarray(inputs["edge"].astype(np.float16).reshape(B, C, N))

    in_maps = []
    for core in range(8):
        b, qs = core // 4, core % 4
        m = dict(shared)
        m["rgb_s"] = np.ascontiguousarray(rgb[b][:, QCH * qs:QCH * (qs + 1)])
        m["edge"] = edge[b]
        in_maps.append(m)

    res = run_bass_kernel_spmd(nc, in_maps, core_ids=list(range(8)))
    full = np.zeros((B, C, N), np.float32)
    for core in range(8):
        b, qs = core // 4, core % 4
        full[b][:, QCH * qs:QCH * (qs + 1)] = res.results[core]["out"]
    return full.reshape(B, C, H, W)


# revision 18
# speedup vs baseline: 1.0127x; 1.0127x over previous
"""Cross-modal attention Trainium2 kernel (v5).

Reference computation (all 1x1 convs + folded eval-mode BN):
  q = BN(Wq @ rgb), k = BN(Wk @ edge), v = BN(Wv @ edge)
  attn = softmax(q^T k) per head; xx = relu(attn @ v); out = BN(Wp @ xx)

Shapes: B=2, C=256, H=W=64 (N=4096), heads=8, key_dim=16, d=32.

Sharding: 8 cores = (batch b in {0,1}) x (query-slice qs in {0..3}, 1024
queries each). Each core computes K/V projections for the full N of its
batch (cheap) and attention + output projection for its query slice; the
host concatenates slices. No collectives.

v5 changes vs the 198us baseline:
  - fp16 input pipeline: rgb/edge and the projection weights ship as
    fp16 (half the DMA bytes -> input wire time ~11us instead of ~21us;
    group 0 was DMA-throttled). fp16's 10-bit mantissa matches f32r
    matmul precision, and all downstream score/exp/AV math is unchanged
    (kk/qq stay f32r from the f32 PSUM).
  - Bias algebra: q-bias and bk.bq are softmax-invariant (constant along
    the key axis) and dropped; the k-bias term bq.k_hat[m] is folded as a
    17th contraction row (kk row 32j+16 = Wk_f^T bq_h projected, qq row
    32j+16 = 1.0 via DMA, re-DMA'd after each projection-copy half).
    Projection PSUM->SBUF moves become pure copies placeable on either
    ACT or DVE.
  - Exp engine pattern retuned for the shifted fixed loads.

Per-core dataflow (scores kept transposed so softmax-sum and the AV
contraction both run on the m axis): see baseline notes. Exp split:
ACT true exp / DVE Schraudolph bitwise 2^x (~+-3% multiplicative error
that cancels in softmax normalization). GPSIMD cannot touch PSUM, so
Pool only carries the SBUF-side per-head xnm scales.

PSUM (8 banks): scp 3 x [128,1024]f32 (2 banks each, depth-3 QKT->exp
rotation; in-stream projection psums transiently borrow scp zones);
avp [128,264]f32; fixt [128,512]f32 (vto double-buffer slots, later
outproj psums and transpose outputs).
"""

import sys

for p in ("/opt/trn_rl_repo", "/opt/trn_rl_repo/concourse"):
    if p not in sys.path:
        sys.path.insert(0, p)

import numpy as np

import concourse.bass as bass
import concourse.mybir as mybir
import concourse.tile as tile
from concourse.bass_utils import run_bass_kernel_spmd

F32 = mybir.dt.float32
F32R = mybir.dt.float32r
F16 = mybir.dt.float16
BF16 = mybir.dt.bfloat16
I16 = mybir.dt.int16
AF = mybir.ActivationFunctionType
ALU = mybir.AluOpType

NUM_HEADS, KD, D = 8, 16, 32
B, C, H, W = 2, 256, 64, 64
N = H * W            # 4096 keys per batch
QCH = 1024           # queries per core
NMT = N // 128       # 32 m-tiles
HB = 33              # per-head AV block: 32 v-cols + 1 denominator col
KC = 17              # contraction rows per head: 16 kd + 1 bias row

# Schraudolph exp: bf16_bits(int16(s*SCH_C1 + SCH_C2)) ~= e^s.
SCH_C1 = 184.66496580927726     # 128 * log2(e)
SCH_C2 = 16248.6                # 127*128 minus mean-zeroing interp bias


def _mk_pat(nA, nD):
    """Evenly interleave nA 'A's and nD 'D's (Bresenham)."""
    out, a, d = [], 0, 0
    for i in range(nA + nD):
        if a * nD <= d * nA and a < nA:
            out.append("A")
            a += 1
        else:
            out.append("D")
            d += 1
    return out


# Per-group exp-engine pattern (32 tiles each). Group 0 skews toward ACT
# (DVE carries the vto adds early on); steady-state 16/16.
GROUP_PATS = [_mk_pat(26, 6)] + [_mk_pat(16, 16)] * 7


def build_nc():
    nc = bass.Bass()

    rgb_s = nc.dram_tensor("rgb_s", [C, QCH], F16, kind="ExternalInput")
    edge = nc.dram_tensor("edge", [C, N], F16, kind="ExternalInput")
    w_qA = nc.dram_tensor("w_qA", [C, 128], F16, kind="ExternalInput")
    w_qB = nc.dram_tensor("w_qB", [C, 128], F16, kind="ExternalInput")
    w_kA = nc.dram_tensor("w_kA", [C, 128], F16, kind="ExternalInput")
    w_kB = nc.dram_tensor("w_kB", [C, 128], F16, kind="ExternalInput")
    w_v = nc.dram_tensor("w_v", [C, 256], F16, kind="ExternalInput")
    w_p = nc.dram_tensor("w_p", [256, C], BF16, kind="ExternalInput")
    b_v = nc.dram_tensor("b_v", [128, 256], F32, kind="ExternalInput")
    b_p = nc.dram_tensor("b_p", [C, 1], F32, kind="ExternalInput")
    ones_f = nc.dram_tensor("ones_f", [4, QCH], F32R, kind="ExternalInput")
    ident = nc.dram_tensor("ident", [128, 128], BF16, kind="ExternalInput")
    out = nc.dram_tensor("out", [C, QCH], F32, kind="ExternalOutput")

    with tile.TileContext(nc) as tc:
        with tc.tile_pool(name="const", bufs=1) as cp, \
             tc.tile_pool(name="data", bufs=1) as dp:
            wq = [cp.tile([128, 256], F16, name=f"wq{x}", tag=f"wq{x}") for x in "AB"]
            wk = [cp.tile([128, 256], F16, name=f"wk{x}", tag=f"wk{x}") for x in "AB"]
            wv = cp.tile([128, 512], F16, name="wv", tag="wv")
            wp = cp.tile([128, 512], BF16, name="wp", tag="wp")
            bv = cp.tile([128, 256], F32, name="bv", tag="bv")
            bp = cp.tile([128, 2], F32, name="bp", tag="bp")
            idn = cp.tile([128, 128], BF16, name="idn", tag="idn")
            zrow = cp.tile([1, 128], BF16, name="zrow", tag="zrow")

            nc.vector.memset(zrow[:], 0.0)

            rgb_sb = dp.tile([128, 2 * QCH], F16, name="rgb_sb", tag="rgb")
            edge_sb = [dp.tile([128, N], F16, name=f"edge{k}", tag=f"edge{k}") for k in range(2)]

            # Input DMA ordering: first-use order; consolidated into few
            # DMAs (each dma_start holds the shared HWDGE device ~0.63us,
            # which -- not wire bandwidth -- bounds the fp16 input phase).
            # First pieces stay small for latency.
            nc.sync.dma_start(
                wk[0][:].rearrange("p (k c) -> p k c", k=2),
                w_kA[:].rearrange("(k p) c -> p k c", k=2))
            nc.scalar.dma_start(
                wq[0][:].rearrange("p (k c) -> p k c", k=2),
                w_qA[:].rearrange("(k p) c -> p k c", k=2))
            for k in range(2):      # first edge columns (kk chunk 0 dep)
                nc.scalar.dma_start(edge_sb[k][:, 0:512], edge[128 * k:128 * (k + 1), 0:512])
            for k in range(2):      # rgb first halves (qc=0 queries)
                nc.scalar.dma_start(rgb_sb[:, QCH * k:QCH * k + 512], rgb_s[128 * k:128 * (k + 1), 0:512])
            for k in range(2):
                nc.sync.dma_start(edge_sb[k][:, 512:1024], edge[128 * k:128 * (k + 1), 512:1024])
            nc.sync.dma_start(
                wv[:].rearrange("p (k c) -> p k c", k=2),
                w_v[:].rearrange("(k p) c -> p k c", k=2))
            nc.sync.dma_start(bv[:], b_v[:])
            nc.sync.dma_start(
                bp[:].unsqueeze(2),
                b_p[:].rearrange("(k p) c -> p k c", k=2))
            for k in range(2):
                nc.sync.dma_start(edge_sb[k][:, 1024:2048], edge[128 * k:128 * (k + 1), 1024:2048])
            for k in range(2):
                nc.sync.dma_start(rgb_sb[:, QCH * k + 512:QCH * (k + 1)], rgb_s[128 * k:128 * (k + 1), 512:1024])
            nc.sync.dma_start(
                wk[1][:].rearrange("p (k c) -> p k c", k=2),
                w_kB[:].rearrange("(k p) c -> p k c", k=2))
            nc.sync.dma_start(
                wq[1][:].rearrange("p (k c) -> p k c", k=2),
                w_qB[:].rearrange("(k p) c -> p k c", k=2))
            for k in range(2):
                nc.sync.dma_start(edge_sb[k][:, 2048:4096], edge[128 * k:128 * (k + 1), 2048:4096])
            nc.sync.dma_start(
                wp[:].rearrange("p (k c) -> p k c", k=2),
                w_p[:].rearrange("(k p) c -> p k c", k=2))
            nc.sync.dma_start(idn[:], ident[:])

            qq = [dp.tile([128, QCH], F32R, name=f"qq{x}", tag=f"qq{x}") for x in "AB"]
            kk = [dp.tile([128, N], F32R, name=f"kk{x}", tag=f"kk{x}") for x in "AB"]
            vto = dp.tile([128, NMT * 8 * HB], BF16, name="vto", tag="vto")
            # denominator columns (33rd of each head block) are constant 1.0
            nc.vector.memset(
                vto[:].rearrange("p (m c) -> p m c", c=HB)[:, :, 32], 1.0)
            outb = [dp.tile([128, QCH], F32, name=f"outb{k}", tag=f"outb{k}") for k in range(2)]
            xxt = [dp.tile([128, 512], BF16, name=f"xxt{k}", tag=f"xxt{k}") for k in range(2)]

            # ---- pools ----
            scp = tc.alloc_tile_pool(name="scp", bufs=3, space="PSUM")
            avp = tc.alloc_tile_pool(name="avp", bufs=1, space="PSUM")
            psv = tc.alloc_tile_pool(name="psv", bufs=1, space="PSUM")
            fixt = psv.tile([128, 512], F32, name="fixt", tag="v")
            ep = tc.alloc_tile_pool(name="exp", bufs=12)
            sp = tc.alloc_tile_pool(name="stg", bufs=6)

            # ACT warmup: keep the first real exp single-wait.
            actw = dp.tile([1, 1], BF16, name="actw", tag="actw")
            nc.scalar.activation(actw[:], zrow[0:1, 0:1], AF.Exp)

            def copy_ps(dst, ps, eng):
                # PSUM->SBUF projection move: ACT Identity or DVE copy.
                if eng == "A":
                    nc.scalar.activation(dst, ps, AF.Identity)
                else:
                    nc.vector.tensor_copy(dst, ps)

            def ones_dma(x, j):
                # rewrite the const-1 rows (32g+16) of qq[x] cols 512j..
                # (the projection copy clobbers all 128 partitions).
                nc.sync.dma_start(
                    qq[x].rearrange("(g p) q -> g p q", p=32)[:, 16:17, 512 * j:512 * (j + 1)],
                    ones_f[:, 512 * j:512 * (j + 1)].unsqueeze(1))

            def proj_qq(x, j, eng, ps=None):
                ps = ps if ps is not None else scp.tile([128, 512], F32, name="ps_q", tag="w")
                for k in range(2):
                    nc.tensor.matmul(
                        ps, wq[x][:, 128 * k:128 * (k + 1)],
                        rgb_sb[:, QCH * k + 512 * j:QCH * k + 512 * (j + 1)],
                        start=(k == 0), stop=(k == 1))
                copy_ps(qq[x][:, 512 * j:512 * (j + 1)], ps, eng)
                ones_dma(x, j)

            def proj_kk(x, j, eng, ps=None):
                ps = ps if ps is not None else scp.tile([128, 512], F32, name="ps_k", tag="w")
                for k in range(2):
                    nc.tensor.matmul(
                        ps, wk[x][:, 128 * k:128 * (k + 1)],
                        edge_sb[k][:, 512 * j:512 * (j + 1)],
                        start=(k == 0), stop=(k == 1))
                copy_ps(kk[x][:, 512 * j:512 * (j + 1)], ps, eng)

            def proj_vto(mt):
                ps = fixt[:, 256 * (mt % 2):256 * (mt % 2) + 256]
                for k in range(2):
                    nc.tensor.matmul(
                        ps, edge_sb[k][:, 128 * mt:128 * (mt + 1)],
                        wv[:, 256 * k:256 * (k + 1)],
                        start=(k == 0), stop=(k == 1))
                nc.vector.tensor_add(
                    vto[:, 8 * HB * mt:8 * HB * (mt + 1)].rearrange(
                        "p (h c) -> p h c", c=HB)[:, :, 0:32],
                    ps.rearrange("p (h c) -> p h c", c=32), bv[:])

            def transposes(x):
                xnm_x = xnm_tiles[x]
                for s in range(4):
                    tp = fixt[:, 64 * s:64 * (s + 1)].bitcast(BF16)
                    nc.tensor.transpose(tp, xnm_x[:, 128 * s:128 * (s + 1)], idn[:])
                    nc.vector.tensor_copy(xxt[x][:, 128 * s:128 * (s + 1)], tp)

            def outproj(qc):
                # ct=1 gets its own scp psum so the two halves pipeline
                # (PE on ct=1 while ACT/DMA drain ct=0) -- matters at the
                # kernel tail.
                q0 = 512 * qc
                for ct in range(2):
                    ps = fixt[:] if ct == 0 else scp.tile(
                        [128, 512], F32, name="ps_o", tag="w")
                    for k in range(2):
                        nc.tensor.matmul(
                            ps, wp[:, 256 * k + 128 * ct:256 * k + 128 * (ct + 1)],
                            xxt[k][:], start=(k == 0), stop=(k == 1))
                    nc.scalar.activation(
                        outb[ct][:, q0:q0 + 512], ps,
                        AF.Identity, bias=bp[:, ct:ct + 1])
                    if qc == 1:
                        nc.sync.dma_start(
                            out[128 * ct:128 * (ct + 1), 512:1024],
                            outb[ct][:, 512:1024])

            # per-mtile PE-stream hooks: {(gi, mt): [fn, ...]}
            hooks = {}

            def add_hook(gi, mt, fn):
                hooks.setdefault((gi, mt), []).append(fn)

            # pre-loop: first kk chunk + first qq half, vto 0-3. The rest
            # of kk[0] rides gi0 hooks (paced by edge arrival, scp psums);
            # kk[1]/qq halves ride later hooks (fixt once vto is done).
            proj_kk(0, 0, "A")
            proj_qq(0, 0, "D")
            for i in range(4):
                proj_vto(i)
            for i in range(28):             # vto(4+i), 4 mtiles ahead of use
                add_hook(0, i, lambda mt=4 + i: proj_vto(mt))
            for c in range(1, 8):           # kk[0] chunk c feeds QKT(mt>=4c)
                add_hook(0, 4 * c - 2, lambda c=c: proj_kk(0, c, "A"))
            add_hook(1, 24, lambda: proj_qq(1, 0, "A", fixt[:]))
            add_hook(3, 10, lambda: proj_qq(0, 1, "A", fixt[:]))
            add_hook(5, 14, lambda: proj_qq(1, 1, "A", fixt[:]))
            add_hook(1, 28, lambda: proj_kk(1, 0, "A", fixt[:]))
            for c in range(1, 8):
                add_hook(2, 4 * c - 2,
                         lambda c=c: proj_kk(1, c, "A", fixt[:]))

            def emit_exp(et, sc, eng):
                if eng == "A":
                    nc.scalar.activation(et, sc, AF.Exp)
                else:
                    nc.vector.tensor_scalar(
                        et.bitcast(I16), sc, SCH_C1, SCH_C2,
                        ALU.mult, ALU.add)

            GROUPS = [(qc, x, pr) for qc in (0, 1) for x in (0, 1) for pr in (0, 1)]
            # deferred PE-side epilogue hooks: group gi's xnm tile (x done
            # at odd gi) is transposed inside group gi+1's stream; the
            # output projection of qc=0 rides in group 4.
            add_hook(2, 10, lambda: transposes(0))
            add_hook(4, 10, lambda: transposes(1))
            add_hook(4, 16, lambda: outproj(0))
            add_hook(6, 10, lambda: transposes(0))

            def out_dma_qc0():
                for ct in range(2):
                    nc.sync.dma_start(out[128 * ct:128 * (ct + 1), 0:512],
                                      outb[ct][:, 0:512])
            add_hook(5, 8, out_dma_qc0)
            LAG = 4
            xnm_tiles = [None, None]
            pend = []       # (emit_av_fn, post_fn_or_None)

            def flush_one():
                fn, post = pend.pop(0)
                fn()
                if post is not None:
                    post()

            for gi, (qc, x, pr) in enumerate(GROUPS):
                q0 = 512 * qc
                pat = GROUP_PATS[gi]
                avt = avp.tile([128, 264], F32, name="av", tag="av")

                def make_av(avt, x, pr, mt, et):
                    def emit_av():
                        for j2 in range(2):
                            h = 4 * x + 2 * pr + j2
                            for s in range(4):
                                nc.tensor.matmul(
                                    avt[:, 66 * s + 33 * j2:66 * s + 33 * j2 + 33],
                                    et[:, 512 * j2 + 128 * s:512 * j2 + 128 * (s + 1)],
                                    vto[:, 8 * HB * mt + HB * h:8 * HB * mt + HB * (h + 1)],
                                    start=False, stop=(mt == NMT - 1))
                    return emit_av

                def make_epilogue(avt, gi, qc, x, pr):
                    def epilogue():
                        xxm = sp.tile([128, 264], F32, name="xxm", tag="xxm")
                        nc.scalar.activation(xxm[:], avt[:], AF.Relu)
                        rden = sp.tile([128, 8], F32, name="rden", tag="rden")
                        nc.vector.reciprocal(
                            rden[:],
                            xxm[:].rearrange("p (g c) -> p g c", c=33)[:, :, 32])
                        if pr == 0:
                            xnm_tiles[x] = sp.tile([128, 512], BF16,
                                                   name="xnm", tag="xnm")
                        xnm_x = xnm_tiles[x]
                        if gi == len(GROUPS) - 1:
                            # tail: DVE is idle; one strided op beats the
                            # serial Pool-launch chain
                            nc.vector.scalar_tensor_tensor(
                                xnm_x[:].rearrange(
                                    "p (s v u c) -> p s v u c",
                                    v=2, u=2, c=32)[:, :, pr, :, :],
                                xxm[:].rearrange(
                                    "p (s u c) -> p s u c",
                                    u=2, c=33)[:, :, :, 0:32],
                                1.0,
                                rden[:].rearrange("p (s u) -> p s u", u=2)
                                    .unsqueeze(3).broadcast_to([128, 4, 2, 32]),
                                ALU.mult, ALU.mult)
                        else:
                            for g in range(8):      # g = 2*s + j2
                                s, j2 = g // 2, g % 2
                                c0 = 128 * s + 64 * pr + 32 * j2
                                nc.gpsimd.tensor_scalar_mul(
                                    xnm_x[:, c0:c0 + 32],
                                    xxm[:, 33 * g:33 * g + 32],
                                    rden[:, g:g + 1])
                    return epilogue

                for mt in range(NMT):
                    for fn in hooks.get((gi, mt), []):
                        fn()
                    if mt == 6:
                        # whole-bank start=True clear of this group's AV
                        # accumulator; deferred here so it lands after the
                        # previous group's epilogue (relu) was emitted.
                        nc.tensor.matmul(avt[:], zrow[:], vto[0:1, 0:264],
                                         start=True, stop=False)
                    limit = 7 if mt < 8 else LAG
                    while len(pend) > limit:
                        flush_one()
                    sc = scp.tile([128, 1024], F32, name="sc", tag="w")
                    for j2 in range(2):
                        j = 2 * pr + j2
                        nc.tensor.matmul(
                            sc[:, 512 * j2:512 * (j2 + 1)],
                            kk[x][32 * j:32 * j + KC, 128 * mt:128 * (mt + 1)],
                            qq[x][32 * j:32 * j + KC, q0:q0 + 512],
                            start=True, stop=True,
                            tile_position=(32 * j, 0))
                    et = ep.tile([128, 1024], BF16, name="et", tag="et")
                    emit_exp(et[:], sc[:], pat[mt])
                    post = make_epilogue(avt, gi, qc, x, pr) if mt == NMT - 1 else None
                    pend.append((make_av(avt, x, pr, mt, et), post))

            # tail: flush remaining AVs (fires the last epilogue), then
            # the final transposes + output projection
            while pend:
                flush_one()
            transposes(1)
            outproj(1)

            for _p in (sp, ep, psv, avp, scp):
                _p.release()

    # walrus codegen accepts only ONE sync wait on compute instructions.
    # Drop self-engine waits (hardware-guaranteed in-order), then hoist
    # any remaining extras into same-engine NoOps.
    _self_prefix = {
        "EngineType.PE": "PE",
        "EngineType.Activation": "Activation",
        "EngineType.DVE": "DVE",
        "EngineType.Pool": "Pool",
        "EngineType.SP": "SP",
    }
    for f in nc.m.functions:
        for bb in f.blocks:
            for inst in bb.instructions:
                si = inst.sync_info
                if si is None or not si.on_wait or len(si.on_wait) < 2:
                    continue
                pref = _self_prefix.get(str(getattr(inst, "engine", "")), None)
                if pref is None:
                    continue
                kept = [w for w in si.on_wait
                        if not str(w.ant_name).startswith(pref)]
                if not kept or len(kept) == len(si.on_wait):
                    continue
                si.on_wait = kept

    uid = [0]
    for f in nc.m.functions:
        for bb in f.blocks:
            new_insts = []
            for inst in bb.instructions:
                si = inst.sync_info
                if si is not None and si.on_wait and len(si.on_wait) > 1:
                    for w in si.on_wait[:-1]:
                        uid[0] += 1
                        nop = mybir.InstNoOp(
                            name=f"I-waitsplit-{uid[0]}", ins=[], outs=[])
                        nop.engine = inst.engine
                        nop.sync_info = mybir.SyncInfo(
                            on_wait=[w], on_update=[])
                        new_insts.append(nop)
                    si.on_wait = [si.on_wait[-1]]
                new_insts.append(inst)
            bb.instructions = new_insts
    return nc


_CACHE = {}


def _prep_host(inputs):
    """Fold BN into weights; build head-split layouts shared by all cores."""
    import ml_dtypes
    f = np.float32
    f16 = np.float16
    Wq = (inputs["Wq"] * inputs["sq"][:, None]).astype(f)
    Wk = (inputs["Wk"] * inputs["sk"][:, None]).astype(f)
    Wv = (inputs["Wv"] * inputs["sv"][:, None]).astype(f)
    Wp = (inputs["Wp"] * inputs["sp"][:, None]).astype(f)
    bq = inputs["bq"].astype(f)

    def split_q(Wt):
        # Wt: [C, 128] (transposed folded Wq). Bias rows dropped
        # (q-bias term is constant along the softmax axis).
        o = []
        for g in range(2):
            Wx = np.zeros((C, 128), f)
            for j in range(4):
                h = 4 * g + j
                Wx[:, 32 * j:32 * j + KD] = Wt[:, KD * h:KD * (h + 1)]
            o.append(np.ascontiguousarray(Wx).astype(f16))
        return o

    def split_k(Wt):
        # col 32j+16 projects rk[m] = bq_h^T k_hat[m] (k-bias term folded
        # as a 17th contraction row; pairs with the const-1 qq row).
        o = []
        for g in range(2):
            Wx = np.zeros((C, 128), f)
            for j in range(4):
                h = 4 * g + j
                Wx[:, 32 * j:32 * j + KD] = Wt[:, KD * h:KD * (h + 1)]
                Wx[:, 32 * j + 16] = Wt[:, KD * h:KD * (h + 1)] @ bq[KD * h:KD * (h + 1)]
            o.append(np.ascontiguousarray(Wx).astype(f16))
        return o

    wqA, wqB = split_q(Wq.T.astype(f))
    wkA, wkB = split_k(Wk.T.astype(f))
    WvT = Wv.T.astype(f)                      # [C, 256] cols (h, d)
    ident_bf16 = np.eye(128, dtype=ml_dtypes.bfloat16)
    return dict(
        w_qA=wqA, w_qB=wqB, w_kA=wkA, w_kB=wkB,
        w_v=np.ascontiguousarray(WvT).astype(f16),
        w_p=np.ascontiguousarray(Wp.T).astype(ml_dtypes.bfloat16),
        b_v=np.ascontiguousarray(np.broadcast_to(inputs["bv"].astype(f), (128, 256))),
        b_p=inputs["bp"].astype(f).reshape(C, 1),
        ones_f=np.ones((4, QCH), f),
        ident=ident_bf16,
    )


def kernel(**inputs) -> np.ndarray:
    inputs = {k: np.asarray(v) for k, v in inputs.items()}
    if "nc" not in _CACHE:
        _CACHE["nc"] = build_nc()
    nc = _CACHE["nc"]

    shared = _prep_host(inputs)
    rgb = np.ascontiguousarray(inputs["rgb"].astype(np.float16).reshape(B, C, N))
    edge = np.ascontiguousarray(inputs["edge"].astype(np.float16).reshape(B, C, N))

    in_maps = []
    for core in range(8):
        b, qs = core // 4, core % 4
        m = dict(shared)
        m["rgb_s"] = np.ascontiguousarray(rgb[b][:, QCH * qs:QCH * (qs + 1)])
        m["edge"] = edge[b]
        in_maps.append(m)

    res = run_bass_kernel_spmd(nc, in_maps, core_ids=list(range(8)))
    full = np.zeros((B, C, N), np.float32)
    for core in range(8):
        b, qs = core // 4, core % 4
        full[b][:, QCH * qs:QCH * (qs + 1)] = res.results[core]["out"]
    return full.reshape(B, C, H, W)


# revision 21
# speedup vs baseline: 1.0136x; 1.0008x over previous
"""Cross-modal attention Trainium2 kernel (v5).

Reference computation (all 1x1 convs + folded eval-mode BN):
  q = BN(Wq @ rgb), k = BN(Wk @ edge), v = BN(Wv @ edge)
  attn = softmax(q^T k) per head; xx = relu(attn @ v); out = BN(Wp @ xx)

Shapes: B=2, C=256, H=W=64 (N=4096), heads=8, key_dim=16, d=32.

Sharding: 8 cores = (batch b in {0,1}) x (query-slice qs in {0..3}, 1024
queries each). Each core computes K/V projections for the full N of its
batch (cheap) and attention + output projection for its query slice; the
host concatenates slices. No collectives.

v5 changes vs the 198us baseline:
  - fp16 input pipeline: rgb/edge and the projection weights ship as
    fp16 (half the DMA bytes -> input wire time ~11us instead of ~21us;
    group 0 was DMA-throttled). fp16's 10-bit mantissa matches f32r
    matmul precision, and all downstream score/exp/AV math is unchanged
    (kk/qq stay f32r from the f32 PSUM).
  - Bias algebra: q-bias and bk.bq are softmax-invariant (constant along
    the key axis) and dropped; the k-bias term bq.k_hat[m] is folded as a
    17th contraction row (kk row 32j+16 = Wk_f^T bq_h projected, qq row
    32j+16 = 1.0 via DMA, re-DMA'd after each projection-copy half).
    Projection PSUM->SBUF moves become pure copies placeable on either
    ACT or DVE.
  - Exp engine pattern retuned for the shifted fixed loads.

Per-core dataflow (scores kept transposed so softmax-sum and the AV
contraction both run on the m axis): see baseline notes. Exp split:
ACT true exp / DVE Schraudolph bitwise 2^x (~+-3% multiplicative error
that cancels in softmax normalization). GPSIMD cannot touch PSUM, so
Pool only carries the SBUF-side per-head xnm scales.

PSUM (8 banks): scp 3 x [128,1024]f32 (2 banks each, depth-3 QKT->exp
rotation; in-stream projection psums transiently borrow scp zones);
avp [128,264]f32; fixt [128,512]f32 (vto double-buffer slots, later
outproj psums and transpose outputs).
"""

import sys

for p in ("/opt/trn_rl_repo", "/opt/trn_rl_repo/concourse"):
    if p not in sys.path:
        sys.path.insert(0, p)

import numpy as np

import concourse.bass as bass
import concourse.mybir as mybir
import concourse.tile as tile
from concourse.bass_utils import run_bass_kernel_spmd

F32 = mybir.dt.float32
F32R = mybir.dt.float32r
F16 = mybir.dt.float16
BF16 = mybir.dt.bfloat16
I16 = mybir.dt.int16
AF = mybir.ActivationFunctionType
ALU = mybir.AluOpType

NUM_HEADS, KD, D = 8, 16, 32
B, C, H, W = 2, 256, 64, 64
N = H * W            # 4096 keys per batch
QCH = 1024           # queries per core
NMT = N // 128       # 32 m-tiles
HB = 33              # per-head AV block: 32 v-cols + 1 denominator col
KC = 17              # contraction rows per head: 16 kd + 1 bias row

# Schraudolph exp: bf16_bits(int16(s*SCH_C1 + SCH_C2)) ~= e^s.
SCH_C1 = 184.66496580927726     # 128 * log2(e)
SCH_C2 = 16248.6                # 127*128 minus mean-zeroing interp bias


def _mk_pat(nA, nD):
    """Evenly interleave nA 'A's and nD 'D's (Bresenham)."""
    out, a, d = [], 0, 0
    for i in range(nA + nD):
        if a * nD <= d * nA and a < nA:
            out.append("A")
            a += 1
        else:
            out.append("D")
            d += 1
    return out


# Per-group exp-engine pattern (32 tiles each), matched to each group's
# fixed engine load: g0 DVE carries the vto adds; g1 ACT carries the
# kk[1] projection copies; g4 ACT carries outproj(0); the tail of g7
# ends on ACT (faster per tile) while DVE does the final epilogue.
GROUP_PATS = [
    _mk_pat(21, 11),            # g0: DVE has vto adds (12.5us)
    _mk_pat(15, 17),            # g1: ACT has kk[1]+qq[1] copies
    _mk_pat(17, 15),
    _mk_pat(17, 15),            # g3: qq(0,1) copy on ACT
    _mk_pat(16, 16),            # g4: outproj(0) bias-moves on ACT
    _mk_pat(17, 15),
    _mk_pat(17, 15),
    _mk_pat(15, 12) + ["A"] * 5,  # g7: finish on ACT
]


def build_nc():
    nc = bass.Bass()

    rgb_s = nc.dram_tensor("rgb_s", [C, QCH], F16, kind="ExternalInput")
    edge = nc.dram_tensor("edge", [C, N], F16, kind="ExternalInput")
    w_qA = nc.dram_tensor("w_qA", [C, 128], F16, kind="ExternalInput")
    w_qB = nc.dram_tensor("w_qB", [C, 128], F16, kind="ExternalInput")
    w_kA = nc.dram_tensor("w_kA", [C, 128], F16, kind="ExternalInput")
    w_kB = nc.dram_tensor("w_kB", [C, 128], F16, kind="ExternalInput")
    w_v = nc.dram_tensor("w_v", [C, 256], F16, kind="ExternalInput")
    w_p = nc.dram_tensor("w_p", [256, C], BF16, kind="ExternalInput")
    b_v = nc.dram_tensor("b_v", [128, 256], F32, kind="ExternalInput")
    b_p = nc.dram_tensor("b_p", [C, 1], F32, kind="ExternalInput")
    ones_f = nc.dram_tensor("ones_f", [4, QCH], F32R, kind="ExternalInput")
    ident = nc.dram_tensor("ident", [128, 128], BF16, kind="ExternalInput")
    out = nc.dram_tensor("out", [C, QCH], F32, kind="ExternalOutput")

    with tile.TileContext(nc) as tc:
        with tc.tile_pool(name="const", bufs=1) as cp, \
             tc.tile_pool(name="data", bufs=1) as dp:
            wq = [cp.tile([128, 256], F16, name=f"wq{x}", tag=f"wq{x}") for x in "AB"]
            wk = [cp.tile([128, 256], F16, name=f"wk{x}", tag=f"wk{x}") for x in "AB"]
            wv = cp.tile([128, 512], F16, name="wv", tag="wv")
            wp = cp.tile([128, 512], BF16, name="wp", tag="wp")
            bv = cp.tile([128, 256], F32, name="bv", tag="bv")
            bp = cp.tile([128, 2], F32, name="bp", tag="bp")
            idn = cp.tile([128, 128], BF16, name="idn", tag="idn")
            zrow = cp.tile([1, 128], BF16, name="zrow", tag="zrow")

            nc.vector.memset(zrow[:], 0.0)

            rgb_sb = dp.tile([128, 2 * QCH], F16, name="rgb_sb", tag="rgb")
            edge_sb = [dp.tile([128, N], F16, name=f"edge{k}", tag=f"edge{k}") for k in range(2)]

            # Input DMA ordering: first-use order; consolidated into few
            # DMAs (each dma_start holds the shared HWDGE device ~0.63us,
            # which -- not wire bandwidth -- bounds the fp16 input phase).
            # First pieces stay small for latency.
            nc.sync.dma_start(
                wk[0][:].rearrange("p (k c) -> p k c", k=2),
                w_kA[:].rearrange("(k p) c -> p k c", k=2))
            nc.scalar.dma_start(
                wq[0][:].rearrange("p (k c) -> p k c", k=2),
                w_qA[:].rearrange("(k p) c -> p k c", k=2))
            for k in range(2):      # first edge columns (kk chunk 0 dep)
                nc.scalar.dma_start(edge_sb[k][:, 0:512], edge[128 * k:128 * (k + 1), 0:512])
            for k in range(2):      # rgb first halves (qc=0 queries)
                nc.scalar.dma_start(rgb_sb[:, QCH * k:QCH * k + 512], rgb_s[128 * k:128 * (k + 1), 0:512])
            for k in range(2):
                nc.sync.dma_start(edge_sb[k][:, 512:1024], edge[128 * k:128 * (k + 1), 512:1024])
            nc.sync.dma_start(
                wv[:].rearrange("p (k c) -> p k c", k=2),
                w_v[:].rearrange("(k p) c -> p k c", k=2))
            nc.sync.dma_start(bv[:], b_v[:])
            nc.sync.dma_start(
                bp[:].unsqueeze(2),
                b_p[:].rearrange("(k p) c -> p k c", k=2))
            for k in range(2):
                nc.sync.dma_start(edge_sb[k][:, 1024:2048], edge[128 * k:128 * (k + 1), 1024:2048])
            for k in range(2):
                nc.sync.dma_start(rgb_sb[:, QCH * k + 512:QCH * (k + 1)], rgb_s[128 * k:128 * (k + 1), 512:1024])
            nc.sync.dma_start(
                wk[1][:].rearrange("p (k c) -> p k c", k=2),
                w_kB[:].rearrange("(k p) c -> p k c", k=2))
            nc.sync.dma_start(
                wq[1][:].rearrange("p (k c) -> p k c", k=2),
                w_qB[:].rearrange("(k p) c -> p k c", k=2))
            for k in range(2):
                nc.sync.dma_start(edge_sb[k][:, 2048:4096], edge[128 * k:128 * (k + 1), 2048:4096])
            nc.sync.dma_start(
                wp[:].rearrange("p (k c) -> p k c", k=2),
                w_p[:].rearrange("(k p) c -> p k c", k=2))
            nc.sync.dma_start(idn[:], ident[:])

            qq = [dp.tile([128, QCH], F32R, name=f"qq{x}", tag=f"qq{x}") for x in "AB"]
            kk = [dp.tile([128, N], F32R, name=f"kk{x}", tag=f"kk{x}") for x in "AB"]
            vto = dp.tile([128, NMT * 8 * HB], BF16, name="vto", tag="vto")
            # denominator columns (33rd of each head block) are constant 1.0
            nc.vector.memset(
                vto[:].rearrange("p (m c) -> p m c", c=HB)[:, :, 32], 1.0)
            outb = [dp.tile([128, QCH], F32, name=f"outb{k}", tag=f"outb{k}") for k in range(2)]
            xxt = [dp.tile([128, 512], BF16, name=f"xxt{k}", tag=f"xxt{k}") for k in range(2)]

            # ---- pools ----
            scp = tc.alloc_tile_pool(name="scp", bufs=3, space="PSUM")
            avp = tc.alloc_tile_pool(name="avp", bufs=1, space="PSUM")
            psv = tc.alloc_tile_pool(name="psv", bufs=1, space="PSUM")
            fixt = psv.tile([128, 512], F32, name="fixt", tag="v")
            ep = tc.alloc_tile_pool(name="exp", bufs=12)
            sp = tc.alloc_tile_pool(name="stg", bufs=6)

            # ACT warmup: keep the first real exp single-wait.
            actw = dp.tile([1, 1], BF16, name="actw", tag="actw")
            nc.scalar.activation(actw[:], zrow[0:1, 0:1], AF.Exp)

            def copy_ps(dst, ps, eng):
                # PSUM->SBUF projection move: ACT Identity or DVE copy.
                if eng == "A":
                    nc.scalar.activation(dst, ps, AF.Identity)
                else:
                    nc.vector.tensor_copy(dst, ps)

            def ones_dma(x, j):
                # rewrite the const-1 rows (32g+16) of qq[x] cols 512j..
                # (the projection copy clobbers all 128 partitions).
                nc.sync.dma_start(
                    qq[x].rearrange("(g p) q -> g p q", p=32)[:, 16:17, 512 * j:512 * (j + 1)],
                    ones_f[:, 512 * j:512 * (j + 1)].unsqueeze(1))

            def proj_qq(x, j, eng, ps=None):
                ps = ps if ps is not None else scp.tile([128, 512], F32, name="ps_q", tag="w")
                for k in range(2):
                    nc.tensor.matmul(
                        ps, wq[x][:, 128 * k:128 * (k + 1)],
                        rgb_sb[:, QCH * k + 512 * j:QCH * k + 512 * (j + 1)],
                        start=(k == 0), stop=(k == 1))
                copy_ps(qq[x][:, 512 * j:512 * (j + 1)], ps, eng)
                ones_dma(x, j)

            def proj_kk(x, j, eng, ps=None):
                ps = ps if ps is not None else scp.tile([128, 512], F32, name="ps_k", tag="w")
                for k in range(2):
                    nc.tensor.matmul(
                        ps, wk[x][:, 128 * k:128 * (k + 1)],
                        edge_sb[k][:, 512 * j:512 * (j + 1)],
                        start=(k == 0), stop=(k == 1))
                copy_ps(kk[x][:, 512 * j:512 * (j + 1)], ps, eng)

            def proj_vto(mt):
                ps = fixt[:, 256 * (mt % 2):256 * (mt % 2) + 256]
                for k in range(2):
                    nc.tensor.matmul(
                        ps, edge_sb[k][:, 128 * mt:128 * (mt + 1)],
                        wv[:, 256 * k:256 * (k + 1)],
                        start=(k == 0), stop=(k == 1))
                nc.vector.tensor_add(
                    vto[:, 8 * HB * mt:8 * HB * (mt + 1)].rearrange(
                        "p (h c) -> p h c", c=HB)[:, :, 0:32],
                    ps.rearrange("p (h c) -> p h c", c=32), bv[:])

            def transposes(x):
                xnm_x = xnm_tiles[x]
                for s in range(4):
                    tp = fixt[:, 64 * s:64 * (s + 1)].bitcast(BF16)
                    nc.tensor.transpose(tp, xnm_x[:, 128 * s:128 * (s + 1)], idn[:])
                    nc.vector.tensor_copy(xxt[x][:, 128 * s:128 * (s + 1)], tp)

            def outproj(qc):
                # ct=1 gets its own scp psum so the two halves pipeline
                # (PE on ct=1 while ACT/DMA drain ct=0) -- matters at the
                # kernel tail.
                q0 = 512 * qc
                for ct in range(2):
                    ps = fixt[:] if ct == 0 else scp.tile(
                        [128, 512], F32, name="ps_o", tag="w")
                    for k in range(2):
                        nc.tensor.matmul(
                            ps, wp[:, 256 * k + 128 * ct:256 * k + 128 * (ct + 1)],
                            xxt[k][:], start=(k == 0), stop=(k == 1))
                    nc.scalar.activation(
                        outb[ct][:, q0:q0 + 512], ps,
                        AF.Identity, bias=bp[:, ct:ct + 1])
                    if qc == 1:
                        nc.sync.dma_start(
                            out[128 * ct:128 * (ct + 1), 512:1024],
                            outb[ct][:, 512:1024])

            # per-mtile PE-stream hooks: {(gi, mt): [fn, ...]}
            hooks = {}

            def add_hook(gi, mt, fn):
                hooks.setdefault((gi, mt), []).append(fn)

            # pre-loop: first kk chunk + first qq half, vto 0-3. The rest
            # of kk[0] rides gi0 hooks (paced by edge arrival, scp psums);
            # kk[1]/qq halves ride later hooks (fixt once vto is done).
            proj_kk(0, 0, "A")
            proj_qq(0, 0, "D")
            for i in range(4):
                proj_vto(i)
            for i in range(28):             # vto(4+i), 4 mtiles ahead of use
                add_hook(0, i, lambda mt=4 + i: proj_vto(mt))
            for c in range(1, 8):           # kk[0] chunk c feeds QKT(mt>=4c)
                add_hook(0, 4 * c - 2, lambda c=c: proj_kk(0, c, "A"))
            add_hook(1, 24, lambda: proj_qq(1, 0, "A", fixt[:]))
            add_hook(3, 10, lambda: proj_qq(0, 1, "A", fixt[:]))
            add_hook(5, 14, lambda: proj_qq(1, 1, "A", fixt[:]))
            # kk[1] chunks all inside gi1 (fixt is free once vto drains);
            # everything is ready well before gi2 starts consuming it.
            for c in range(8):
                add_hook(1, 4 * c, lambda c=c: proj_kk(1, c, "A", fixt[:]))

            def emit_exp(et, sc, eng):
                if eng == "A":
                    nc.scalar.activation(et, sc, AF.Exp)
                else:
                    nc.vector.tensor_scalar(
                        et.bitcast(I16), sc, SCH_C1, SCH_C2,
                        ALU.mult, ALU.add)

            GROUPS = [(qc, x, pr) for qc in (0, 1) for x in (0, 1) for pr in (0, 1)]
            # deferred PE-side epilogue hooks: group gi's xnm tile (x done
            # at odd gi) is transposed inside group gi+1's stream; the
            # output projection of qc=0 rides in group 4.
            add_hook(2, 10, lambda: transposes(0))
            add_hook(4, 10, lambda: transposes(1))
            add_hook(4, 16, lambda: outproj(0))
            add_hook(6, 10, lambda: transposes(0))

            def out_dma_qc0():
                for ct in range(2):
                    nc.sync.dma_start(out[128 * ct:128 * (ct + 1), 0:512],
                                      outb[ct][:, 0:512])
            add_hook(5, 8, out_dma_qc0)
            LAG = 4
            xnm_tiles = [None, None]
            pend = []       # (emit_av_fn, post_fn_or_None)

            def flush_one():
                fn, post = pend.pop(0)
                fn()
                if post is not None:
                    post()

            for gi, (qc, x, pr) in enumerate(GROUPS):
                q0 = 512 * qc
                pat = GROUP_PATS[gi]
                avt = avp.tile([128, 264], F32, name="av", tag="av")

                def make_av(avt, x, pr, mt, et):
                    def emit_av():
                        for j2 in range(2):
                            h = 4 * x + 2 * pr + j2
                            for s in range(4):
                                nc.tensor.matmul(
                                    avt[:, 66 * s + 33 * j2:66 * s + 33 * j2 + 33],
                                    et[:, 512 * j2 + 128 * s:512 * j2 + 128 * (s + 1)],
                                    vto[:, 8 * HB * mt + HB * h:8 * HB * mt + HB * (h + 1)],
                                    start=False, stop=(mt == NMT - 1))
                    return emit_av

                def make_epilogue(avt, gi, qc, x, pr):
                    def epilogue():
                        xxm = sp.tile([128, 264], F32, name="xxm", tag="xxm")
                        nc.scalar.activation(xxm[:], avt[:], AF.Relu)
                        rden = sp.tile([128, 8], F32, name="rden", tag="rden")
                        nc.vector.reciprocal(
                            rden[:],
                            xxm[:].rearrange("p (g c) -> p g c", c=33)[:, :, 32])
                        if pr == 0:
                            xnm_tiles[x] = sp.tile([128, 512], BF16,
                                                   name="xnm", tag="xnm")
                        xnm_x = xnm_tiles[x]
                        if gi == len(GROUPS) - 1:
                            # tail: DVE is idle; one strided op beats the
                            # serial Pool-launch chain
                            nc.vector.scalar_tensor_tensor(
                                xnm_x[:].rearrange(
                                    "p (s v u c) -> p s v u c",
                                    v=2, u=2, c=32)[:, :, pr, :, :],
                                xxm[:].rearrange(
                                    "p (s u c) -> p s u c",
                                    u=2, c=33)[:, :, :, 0:32],
                                1.0,
                                rden[:].rearrange("p (s u) -> p s u", u=2)
                                    .unsqueeze(3).broadcast_to([128, 4, 2, 32]),
                                ALU.mult, ALU.mult)
                        else:
                            for g in range(8):      # g = 2*s + j2
                                s, j2 = g // 2, g % 2
                                c0 = 128 * s + 64 * pr + 32 * j2
                                nc.gpsimd.tensor_scalar_mul(
                                    xnm_x[:, c0:c0 + 32],
                                    xxm[:, 33 * g:33 * g + 32],
                                    rden[:, g:g + 1])
                    return epilogue

                for mt in range(NMT):
                    for fn in hooks.get((gi, mt), []):
                        fn()
                    if mt == 6:
                        # whole-bank start=True clear of this group's AV
                        # accumulator; deferred here so it lands after the
                        # previous group's epilogue (relu) was emitted.
                        nc.tensor.matmul(avt[:], zrow[:], vto[0:1, 0:264],
                                         start=True, stop=False)
                    limit = 7 if mt < 8 else LAG
                    if gi == len(GROUPS) - 1 and mt >= 28:
                        limit = 2    # drain eagerly at the kernel tail
                    while len(pend) > limit:
                        flush_one()
                    sc = scp.tile([128, 1024], F32, name="sc", tag="w")
                    for j2 in range(2):
                        j = 2 * pr + j2
                        nc.tensor.matmul(
                            sc[:, 512 * j2:512 * (j2 + 1)],
                            kk[x][32 * j:32 * j + KC, 128 * mt:128 * (mt + 1)],
                            qq[x][32 * j:32 * j + KC, q0:q0 + 512],
                            start=True, stop=True,
                            tile_position=(32 * j, 0))
                    et = ep.tile([128, 1024], BF16, name="et", tag="et")
                    emit_exp(et[:], sc[:], pat[mt])
                    post = make_epilogue(avt, gi, qc, x, pr) if mt == NMT - 1 else None
                    pend.append((make_av(avt, x, pr, mt, et), post))

            # tail: flush remaining AVs (fires the last epilogue), then
            # the final transposes + output projection
            while pend:
                flush_one()
            transposes(1)
            outproj(1)

            for _p in (sp, ep, psv, avp, scp):
                _p.release()

    # walrus codegen accepts only ONE sync wait on compute instructions.
    # Drop self-engine waits (hardware-guaranteed in-order), then hoist
    # any remaining extras into same-engine NoOps.
    _self_prefix = {
        "EngineType.PE": "PE",
        "EngineType.Activation": "Activation",
        "EngineType.DVE": "DVE",
        "EngineType.Pool": "Pool",
        "EngineType.SP": "SP",
    }
    for f in nc.m.functions:
        for bb in f.blocks:
            for inst in bb.instructions:
                si = inst.sync_info
                if si is None or not si.on_wait or len(si.on_wait) < 2:
                    continue
                pref = _self_prefix.get(str(getattr(inst, "engine", "")), None)
                if pref is None:
                    continue
                kept = [w for w in si.on_wait
                        if not str(w.ant_name).startswith(pref)]
                if not kept or len(kept) == len(si.on_wait):
                    continue
                si.on_wait = kept

    uid = [0]
    for f in nc.m.functions:
        for bb in f.blocks:
            new_insts = []
            for inst in bb.instructions:
                si = inst.sync_info
                if si is not None and si.on_wait and len(si.on_wait) > 1:
                    for w in si.on_wait[:-1]:
                        uid[0] += 1
                        nop = mybir.InstNoOp(
                            name=f"I-waitsplit-{uid[0]}", ins=[], outs=[])
                        nop.engine = inst.engine
                        nop.sync_info = mybir.SyncInfo(
                            on_wait=[w], on_update=[])
                        new_insts.append(nop)
                    si.on_wait = [si.on_wait[-1]]
                new_insts.append(inst)
            bb.instructions = new_insts
    return nc


_CACHE = {}


def _prep_host(inputs):
    """Fold BN into weights; build head-split layouts shared by all cores."""
    import ml_dtypes
    f = np.float32
    f16 = np.float16
    Wq = (inputs["Wq"] * inputs["sq"][:, None]).astype(f)
    Wk = (inputs["Wk"] * inputs["sk"][:, None]).astype(f)
    Wv = (inputs["Wv"] * inputs["sv"][:, None]).astype(f)
    Wp = (inputs["Wp"] * inputs["sp"][:, None]).astype(f)
    bq = inputs["bq"].astype(f)

    def split_q(Wt):
        # Wt: [C, 128] (transposed folded Wq). Bias rows dropped
        # (q-bias term is constant along the softmax axis).
        o = []
        for g in range(2):
            Wx = np.zeros((C, 128), f)
            for j in range(4):
                h = 4 * g + j
                Wx[:, 32 * j:32 * j + KD] = Wt[:, KD * h:KD * (h + 1)]
            o.append(np.ascontiguousarray(Wx).astype(f16))
        return o

    def split_k(Wt):
        # col 32j+16 projects rk[m] = bq_h^T k_hat[m] (k-bias term folded
        # as a 17th contraction row; pairs with the const-1 qq row).
        o = []
        for g in range(2):
            Wx = np.zeros((C, 128), f)
            for j in range(4):
                h = 4 * g + j
                Wx[:, 32 * j:32 * j + KD] = Wt[:, KD * h:KD * (h + 1)]
                Wx[:, 32 * j + 16] = Wt[:, KD * h:KD * (h + 1)] @ bq[KD * h:KD * (h + 1)]
            o.append(np.ascontiguousarray(Wx).astype(f16))
        return o

    wqA, wqB = split_q(Wq.T.astype(f))
    wkA, wkB = split_k(Wk.T.astype(f))
    WvT = Wv.T.astype(f)                      # [C, 256] cols (h, d)
    ident_bf16 = np.eye(128, dtype=ml_dtypes.bfloat16)
    return dict(
        w_qA=wqA, w_qB=wqB, w_kA=wkA, w_kB=wkB,
        w_v=np.ascontiguousarray(WvT).astype(f16),
        w_p=np.ascontiguousarray(Wp.T).astype(ml_dtypes.bfloat16),
        b_v=np.ascontiguousarray(np.broadcast_to(inputs["bv"].astype(f), (128, 256))),
        b_p=inputs["bp"].astype(f).reshape(C, 1),
        ones_f=np.ones((4, QCH), f),
        ident=ident_bf16,
    )


def kernel(**inputs) -> np.ndarray:
    inputs = {k: np.asarray(v) for k, v in inputs.items()}
    if "nc" not in _CACHE:
        _CACHE["nc"] = build_nc()
    nc = _CACHE["nc"]

    shared = _prep_host(inputs)
    rgb = np.ascontiguousarray(inputs["rgb"].astype(np.float16).reshape(B, C, N))
    edge = np.ascontiguousarray(inputs["edge"].astype(np.float16).reshape(B, C, N))

    in_maps = []
    for core in range(8):
        b, qs = core // 4, core % 4
        m = dict(shared)
        m["rgb_s"] = np.ascontiguousarray(rgb[b][:, QCH * qs:QCH * (qs + 1)])
        m["edge"] = edge[b]
        in_maps.append(m)

    res = run_bass_kernel_spmd(nc, in_maps, core_ids=list(range(8)))
    full = np.zeros((B, C, N), np.float32)
    for core in range(8):
        b, qs = core // 4, core % 4
        full[b][:, QCH * qs:QCH * (qs + 1)] = res.results[core]["out"]
    return full.reshape(B, C, H, W)
